# revision 31
# baseline (speedup 1.0000x reference)
"""Trainium2 Bass kernel for ContinuousBinaryTreeConvLayer.

Math (per batch b, node n, child slot j in [0,8)):
  m_j   = (children[n,j] != 0)
  s     = sum_j m_j
  H_r[n] = sum_j cr_j * Z[c_j],  S_m[n] = sum_j m_j * Z[c_j]
  out_n = relu(Z[n] @ w_t + H_r @ (w_r - w_l) + S_m @ w_l + bias)

with cr_j = j*m_j/(s-1) for s>=2, cr = 0.5*m_0 at j=0 for s==1, else 0.

Implementation: the child gather/aggregation is NOT a dma_gather (the SWDGE
descriptor generation on the Q7 cores is the bottleneck at ~8 ns/index =
517 us/core for 65536 indices).  Instead the aggregation is expressed as a
dense matmul against host-built window coefficient matrices:

  aggT[f, n] = sum_t  Z_t^T @ C_t[., n]      (t = 16 windows of 128 source
                                              rows, PSUM-accumulated)

where C_t[i, n] = sum of coefficients of slots (n, j) with children[n,j] ==
128*t + i.  C is pure graph-structure preprocessing of `children` (like the
baseline's gather-index relayout), shipped over *affine* DMA at full HBM
bandwidth.  The cr coefficients are factored as cr = scale_n * (j*m_j) so
every C entry ({0, 0.5, 1..7}) is exactly representable in fp16/fp8; the
per-node scale 1/(s-1) is applied on DVE in stage 2 (exact algebra).

Per core (data-parallel over batch: 4 batches/core x 8 cores), per batch:
  - DMA Z (row-major, fp16) and Z^T (host-transposed, fp16).
  - Stage 1 (PE): 2 halves (cr | m) x 16 windows x 4 bank-matmuls
    [K=128, M=128f, N=512] accumulating aggT in PSUM; evacuate to SBUF fp16.
  - Stage 2 (PE): per 128-node chunk: ps_main = Z^T@w_t + aggT_m@w_l + bias,
    ps_cr = aggT_cr@(w_r-w_l); DVE fuses scale*ps_cr + ps_main; ACT relu;
    DMA out.
"""

import numpy as np

B, N, C, F, O = 32, 2048, 8, 128, 128
NCORES = 8
BPC = B // NCORES            # batches per core
NWIN = N // 128              # 16 source windows per batch
NBANK = 4                    # 512-col matmuls per 2048-col half

_COMPILED = {}

C_DTYPE = "float8e4"         # coefficient matrix dtype: float16 or float8e4


def _build_nc():
    from contextlib import ExitStack

    import concourse.bacc as bacc
    import concourse.mybir as mybir

    import concourse.tile as tile

    dt = mybir.dt
    Alu = mybir.AluOpType
    cdt = getattr(dt, C_DTYPE)

    nc = bacc.Bacc("TRN2", target_bir_lowering=False, debug=False,
                   num_devices=NCORES)

    z_d = nc.dram_tensor("z", [BPC, N, F], dt.float16, kind="ExternalInput")
    zt_d = nc.dram_tensor("zt", [BPC, F, N], dt.float16, kind="ExternalInput")
    ccr_d = nc.dram_tensor("ccr", [BPC, NWIN // 2, 128, 2 * N], cdt,
                           kind="ExternalInput")
    cm_d = nc.dram_tensor("cm", [BPC, NWIN // 2, 128, 2 * N], cdt,
                          kind="ExternalInput")
    scl_d = nc.dram_tensor("scl", [BPC, 128, N], dt.float16,
                           kind="ExternalInput")
    wt_d = nc.dram_tensor("w_t", [F, O], dt.float16, kind="ExternalInput")
    wrl_d = nc.dram_tensor("w_rl", [F, O], dt.float16, kind="ExternalInput")
    wl_d = nc.dram_tensor("w_l", [F, O], dt.float16, kind="ExternalInput")
    b_d = nc.dram_tensor("bias", [1, O], dt.float16, kind="ExternalInput")
    out_d = nc.dram_tensor("out", [BPC, N, O], dt.float16,
                           kind="ExternalOutput")

    with tile.TileContext(nc) as tc, ExitStack() as ctx:
        const_pool = ctx.enter_context(tc.tile_pool(name="consts", bufs=1))
        wpool = ctx.enter_context(tc.tile_pool(name="weights", bufs=1))
        zpool = ctx.enter_context(tc.tile_pool(name="z", bufs=2))
        ztpool = ctx.enter_context(tc.tile_pool(name="zt", bufs=2))
        cpool = ctx.enter_context(tc.tile_pool(name="cmat", bufs=8))
        sclpool = ctx.enter_context(tc.tile_pool(name="scl", bufs=2))
        aggpool = ctx.enter_context(tc.tile_pool(name="aggsb", bufs=2))
        opool = ctx.enter_context(tc.tile_pool(name="ostage", bufs=2))
        aggps = ctx.enter_context(
            tc.tile_pool(name="aggps", bufs=1, space="PSUM"))
        ps2pool = ctx.enter_context(
            tc.tile_pool(name="ps2", bufs=2, space="PSUM"))

        # bmask[k, p] = 1.0 iff k == 0 (row-0 selector for the bias matmul)
        bmask = const_pool.tile([128, 128], dt.float16)
        nc.gpsimd.memset(bmask[:], 1.0)
        nc.gpsimd.affine_select(out=bmask[:], in_=bmask[:],
                                compare_op=Alu.is_equal, fill=0.0, base=0,
                                pattern=[[0, 128]], channel_multiplier=1)

        wt_sb = wpool.tile([F, O], dt.float16)
        wrl_sb = wpool.tile([F, O], dt.float16)
        wl_sb = wpool.tile([F, O], dt.float16)
        bmat = wpool.tile([128, O], dt.float16)
        nc.vector.memset(bmat[:], 0.0)
        nc.sync.dma_start(wt_sb[:], wt_d.ap())
        nc.sync.dma_start(wrl_sb[:], wrl_d.ap())
        nc.sync.dma_start(wl_sb[:], wl_d.ap())
        nc.sync.dma_start(bmat[0:1, :], b_d.ap())

        for b in range(BPC):
            # z_sb[p, (t f)] = nodes[b, 16p + t, f]; "window" t = row set
            # {n : n % 16 == t} with local index i = n // 16 (host C build
            # uses the same (t, i) = (c % 16, c // 16) decomposition).
            z_sb = zpool.tile([128, NWIN * F], dt.float16)
            nc.gpsimd.dma_start(
                z_sb[:], z_d.ap()[b].rearrange("(p t) f -> p (t f)", p=128))
            zt_sb = ztpool.tile([128, N], dt.float16)
            nc.gpsimd.dma_start(zt_sb[:], zt_d.ap()[b])
            scl_sb = sclpool.tile([128, N], dt.float16)
            nc.gpsimd.dma_start(scl_sb[:], scl_d.ap()[b])

            # ---- stage 1: window-routed aggregation ---------------------
            agg_sb = aggpool.tile([128, 2 * N], dt.float16)
            for half, c_d in enumerate((ccr_d, cm_d)):
                ps = [aggps.tile([128, 512], dt.float32, name=f"aggb{k}")
                      for k in range(NBANK)]
                for u in range(NWIN // 2):
                    c_sb = cpool.tile([128, 2 * N], cdt)
                    dma_q = nc.sync if u % 2 == 0 else nc.scalar
                    dma_q.dma_start(c_sb[:], c_d.ap()[b, u])
                    for t in (2 * u, 2 * u + 1):
                        off = (t % 2) * N
                        for k in range(NBANK):
                            nc.tensor.matmul(
                                ps[k][:],
                                z_sb[:, t * F:(t + 1) * F],
                                c_sb[:, off + k * 512:off + (k + 1) * 512],
                                start=(t == 0), stop=(t == NWIN - 1))
                for k in range(NBANK):
                    dst = agg_sb[:, half * N + k * 512:half * N + (k + 1) * 512]
                    if half == 0:
                        # fold the per-node 1/(s-1) scale into the cr half
                        nc.vector.tensor_tensor(
                            dst, ps[k][:], scl_sb[:, k * 512:(k + 1) * 512],
                            op=Alu.mult)
                    else:
                        nc.scalar.copy(dst, ps[k][:])

            # ---- stage 2: output GEMM + scale/bias/relu -----------------
            ost = None
            for c in range(16):
                if c % 4 == 0:
                    ost = opool.tile([128, 512], dt.float16)
                ps2 = ps2pool.tile([128, 512], dt.float32)
                ps_main = ps2[:, 0:128]
                nc.tensor.matmul(ps_main, zt_sb[:, 128 * c:128 * (c + 1)],
                                 wt_sb[:], start=True, stop=False)
                nc.tensor.matmul(ps_main, agg_sb[:, N + 128 * c:N + 128 * (c + 1)],
                                 wl_sb[:], start=False, stop=False)
                nc.tensor.matmul(ps_main, agg_sb[:, 128 * c:128 * (c + 1)],
                                 wrl_sb[:], start=False, stop=False)
                nc.tensor.matmul(ps_main, bmask[:], bmat[:],
                                 start=False, stop=True)
                nc.scalar.activation(ost[:, (c % 4) * 128:(c % 4 + 1) * 128],
                                     ps_main,
                                     mybir.ActivationFunctionType.Relu)
                if c % 4 == 3:
                    q = c // 4
                    nc.gpsimd.dma_start(
                        out_d.ap()[b, 512 * q:512 * (q + 1), :]
                        .rearrange("(sub p) f -> p sub f", p=128),
                        ost[:].rearrange("p (sub f) -> p sub f", f=F))

    nc.compile()
    return nc


def _get_compiled():
    if "nc" not in _COMPILED:
        _COMPILED["nc"] = _build_nc()
    return _COMPILED["nc"]


def _np_cdtype():
    if C_DTYPE == "float16":
        return np.float16
    import ml_dtypes
    return ml_dtypes.float8_e4m3


def _prep_core(nodes_core, children_core, wt16, wrl16, wl16, b16):
    """Host-side prep for one core: fp16 node tables + window coefficient
    matrices (pure index/graph preprocessing of `children`)."""
    cdt = _np_cdtype()
    z16 = np.ascontiguousarray(nodes_core.astype(np.float16))
    zt16 = np.ascontiguousarray(z16.transpose(0, 2, 1))

    ccr = np.empty((BPC, NWIN // 2, 128, 2 * N), dtype=cdt)
    cm = np.empty((BPC, NWIN // 2, 128, 2 * N), dtype=cdt)
    scl = np.empty((BPC, 128, N), dtype=np.float16)
    cols = np.repeat(np.arange(N, dtype=np.int64), C)
    jj = np.arange(C, dtype=np.float64)[None, :]
    for b in range(BPC):
        ch = children_core[b]
        m = (ch != 0).astype(np.float64)
        s = m.sum(1)
        single = s == 1.0
        crw = jj * m
        crw[single, :] = 0.0
        crw[single, 0] = 0.5 * m[single, 0]
        src = ch.astype(np.int64).ravel()
        # (t, i) = (c % 16, c // 16) matches the device z_sb window layout
        flat = ((src % NWIN) * 128 + src // NWIN) * N + cols
        # [NWIN, 128, N] -> window pairs concatenated along the free dim
        ccr[b] = (np.bincount(flat, weights=crw.ravel(), minlength=N * N)
                  .reshape(NWIN // 2, 2, 128, N).transpose(0, 2, 1, 3)
                  .reshape(NWIN // 2, 128, 2 * N))
        cm[b] = (np.bincount(flat, weights=m.ravel(), minlength=N * N)
                 .reshape(NWIN // 2, 2, 128, N).transpose(0, 2, 1, 3)
                 .reshape(NWIN // 2, 128, 2 * N))
        sc = np.ones(N, np.float32)
        big = s >= 2.0
        sc[big] = 1.0 / (s[big] - 1.0)
        scl[b] = np.broadcast_to(sc.astype(np.float16)[None, :], (128, N))
    return {
        "z": z16, "zt": zt16, "ccr": ccr, "cm": cm, "scl": scl,
        "w_t": wt16, "w_rl": wrl16, "w_l": wl16, "bias": b16,
    }


def make_in_maps(nodes, children, w_t, w_l, w_r, b):
    nodes = np.asarray(nodes, dtype=np.float32)
    children = np.asarray(children, dtype=np.int32)
    wt16 = np.asarray(w_t, dtype=np.float32).astype(np.float16)
    wrl16 = (np.asarray(w_r, dtype=np.float32)
             - np.asarray(w_l, dtype=np.float32)).astype(np.float16)
    wl16 = np.asarray(w_l, dtype=np.float32).astype(np.float16)
    b16 = np.asarray(b, dtype=np.float32).astype(np.float16).reshape(1, O)
    in_maps = []
    for core in range(NCORES):
        sl = slice(core * BPC, (core + 1) * BPC)
        in_maps.append(_prep_core(nodes[sl], children[sl],
                                  wt16, wrl16, wl16, b16))
    return in_maps


def kernel(nodes, children, w_t, w_l, w_r, b):
    from concourse.bass_utils import run_bass_kernel_spmd

    nc = _get_compiled()
    in_maps = make_in_maps(nodes, children, w_t, w_l, w_r, b)
    res = run_bass_kernel_spmd(nc, in_maps, core_ids=list(range(NCORES)))
    out = np.concatenate([res.results[c]["out"] for c in range(NCORES)],
                         axis=0)
    return out.astype(np.float32)
